# revision 51
# baseline (speedup 1.0000x reference)
# Trainium2 Bass kernel for nn_MultiHeadAttention_87024627352037.
#
# Full module: y = LayerNorm(x_q + (softmax(mask(QK^T/sqrt(nd))) V) Wo^T + bo)
# with Q/K/V projections of x_q/x_k/x_v. Shapes: B=2, S=2048, D=1024, H=16.
#
# Sharding (8 cores): core c = (batch b=c//4, head-quad g=c%4).
# Each core projects Q/K/V for its 4 heads (dv=256) over its batch and runs
# causal attention in a fully transposed layout (scoresT = K_T^T Q_T, no
# max-subtraction -- scores are O(1); softmax denominator via a ones-column
# in the PV matmul). Projections are streamed and interleaved with the
# attention q-tiles (processed 0,2,4,6,1,3,5,7) so the PE ramps early and
# stays busy.
#
# Scores matmuls for the two heads of a partition pair (rows 0-63 / 64-127)
# are issued adjacently so the PE runs them CONCURRENTLY in different row
# groups (tile_position auto-derived from base partitions); one Exp covers
# both heads' PSUM banks. The softmax denominator reciprocal runs on DVE
# straight out of PSUM and is broadcast across partitions by the (otherwise
# idle) GPSIMD engine -- the ACT engine runs *only* Exp and the PE runs only
# real matmuls. PV matmuls are software-pipelined one step behind the
# scores/exp so the strict-FIFO PE queue never head-of-line blocks on ACT.
# A per-batch AllToAll (groups of 8) re-shards ctx from head-sharding to
# row-sharding; each core computes output projection + residual + LayerNorm
# for its 512 rows. The host only slices, transposes, and concatenates
# numpy arrays.
import os
import sys
import types

import numpy as np

B, S, D, H = 2, 2048, 1024, 16
ND = D // H          # 64
NC = 8               # cores
HPC = H // 4         # 4 heads per core
DVC = HPC * ND       # 256 dv per core
QT = 256             # q tile
NQT = S // QT        # 8 q tiles
KB = 128             # k block
EPS = 1e-5
SCALE = 1.0 / np.sqrt(ND)

# iteration order: even tiles first so the even-parity AllToAll can fire at
# ~44% of the attention work and overlap the odd-tile compute.
ITERS = (0, 2, 4, 6, 1, 3, 5, 7)
# K/V 128-blocks projected at each iteration (front-loaded so tile t always
# has K/V blocks 0..2t+1 available).
KV_SCHED = {0: (0, 1), 2: (2, 3, 4, 5), 4: (6, 7, 8, 9), 6: (10, 11, 12, 13),
            1: (14, 15), 3: (), 5: (), 7: ()}

_cache = {}


def _install_ntff_shim():
    # antenv.axon_hooks is absent in this image; register the NTFF profile
    # hook so trace=True can capture HW exec time (harmless if unused).
    if "antenv.axon_hooks" in sys.modules:
        return
    mod = types.ModuleType("antenv.axon_hooks")
    mod._hook = None
    mod.set_axon_ntff_profile_hook = lambda h: setattr(mod, "_hook", h)
    mod.get_axon_ntff_profile_hook = lambda: mod._hook
    sys.modules["antenv.axon_hooks"] = mod
    try:
        import antenv

        antenv.axon_hooks = mod
        from trn_agent_boot.trn_boot import _ntff_profile_via_ctypes

        mod._hook = _ntff_profile_via_ctypes("/opt/axon/libaxon_pjrt.so")
    except Exception:
        pass


def _build():
    import concourse.bass as bass
    import concourse.mybir as mybir
    import concourse.tile as tile
    from concourse import bacc

    F32 = mybir.dt.float32
    F32R = mybir.dt.float32r
    BF16 = mybir.dt.bfloat16
    FP8 = mybir.dt.float8e4
    I32 = mybir.dt.int32
    ADD = mybir.AluOpType.add
    MUL = mybir.AluOpType.mult
    SUB = mybir.AluOpType.subtract
    SHR = mybir.AluOpType.logical_shift_right
    AF = mybir.ActivationFunctionType

    nc = bacc.Bacc("TRN2", target_bir_lowering=False, debug=False, num_devices=NC)

    def din(name, shape, dt=BF16):
        return nc.dram_tensor(name, shape, dt, kind="ExternalInput").ap()

    # host pre-shuffled layouts: partition-major so every DMA is contiguous
    # 4KB+ per partition (8x fewer descriptors than (c p)->p c rearranges).
    xtq = din("xtq", [128, 8, 8, 256])   # [p, qtile, cblk, col]
    xtk = din("xtk", [128, 8, 8, 256])
    xtv = din("xtv", [128, 8, 8, 256])
    wqT = din("wqT", [128, 8, DVC])
    wkT = din("wkT", [128, 8, DVC])
    wvT = din("wvT", [128, 8, DVC])
    woT = din("woT", [128, 8, D])
    smallc = din("smallc", [128, 288], F32)   # bq2|bk2|eps|pad|bv4x64(@16)
    gam_bc = din("gam_bc", [128, D], F32)
    bet_bc = din("bet_bc", [128, D], F32)
    resid = din("resid", [512, D], F32)       # x_q rows + bo (host pre-added)
    mo8_in = din("mo8", [128, 4 * QT], mybir.dt.float8e4)  # diag mask x2 heads
    ones_r = din("ones_r", [1, 64], F32R)
    out_d = nc.dram_tensor("out", [512, D], F32, kind="ExternalOutput").ap()

    groups = [list(range(NC))]

    with nc.allow_low_precision(reason="f32r/bf16 matmul operand chain"), tile.TileContext(
        nc
    ) as tc:
        with (
            tc.tile_pool(name="const", bufs=1) as cpool,
            tc.tile_pool(name="res", bufs=1) as rpool,
            tc.tile_pool(name="xt", bufs=8) as xtpool,
            tc.tile_pool(name="xt5", bufs=2) as xtpool5,
            tc.tile_pool(name="pt", bufs=6) as ptpool,
            tc.tile_pool(name="dn", bufs=3) as dnpool,
            tc.tile_pool(name="gath", bufs=1) as gathpool,
            tc.tile_pool(name="ln", bufs=2) as lnpool,
            tc.tile_pool(name="ps_s", bufs=2, space="PSUM") as pss,
            tc.tile_pool(name="ps_ctx", bufs=2, space="PSUM") as psc,
            tc.tile_pool(name="ps_m", bufs=2, space="PSUM") as psm,
            tc.tile_pool(name="dram", bufs=1, space="DRAM") as dram,
        ):
            # ---- small constants + projection weights (needed first) ----
            smallc_sb = cpool.tile([128, 288], F32)
            mo8_sb = cpool.tile([128, 4 * QT], FP8)
            wq_sb = cpool.tile([128, 8, DVC], BF16)
            wk_sb = cpool.tile([128, 8, DVC], BF16)
            wv_sb = cpool.tile([128, 8, DVC], BF16)
            # spread startup loads across engine queues so issue overlaps
            nc.sync.dma_start(wk_sb[:], wkT)
            nc.scalar.dma_start(wq_sb[:], wqT)
            nc.gpsimd.dma_start(wv_sb[:], wvT)
            nc.sync.dma_start(smallc_sb[:], smallc)
            ones_sb = cpool.tile([1, 64], F32R)
            nc.sync.dma_start(ones_sb[:], ones_r)
            nc.sync.dma_start(mo8_sb[:], mo8_in)
            bq_sb = smallc_sb[:, 0:2]
            bk_sb = smallc_sb[:, 2:4]
            eps_sb = smallc_sb[:, 12:13]
            nbias_sb = smallc_sb[:, 13:14]  # -2.0 exp bias
            bv_sb = smallc_sb[:, 16:16 + DVC]   # bv broadcast (no ones col)

            # ---- resident activation tensors ----
            QT_sb = rpool.tile([128, 2, S], BF16)   # q^T: [dd(2x128), q]
            KT_sb = rpool.tile([128, 2, S], BF16)   # k^T: [dd(2x128), kpos]
            V_sb = rpool.tile([128, S // 128, HPC * (ND + 4)], FP8)
            ctx_sb = rpool.tile([128, 2, S], BF16)  # ctx^T: [dv(2x128), q]
            # ones columns of the V slots (denominator trick), set once;
            # slots are 68 wide (16B-aligned strides for dual-fp8 ldweights):
            # 64 data cols, a ones col, 3 zero pad cols. Data cols are fully
            # written by proj_v before use -- only pad+ones need memset.
            v4 = V_sb[:].rearrange("p c (h x) -> p c h x", x=ND + 4)
            nc.gpsimd.memset(v4[:, :, :, ND:ND + 4], 0.0)
            nc.gpsimd.memset(v4[:, :, :, ND:ND + 1], 1.0)

            # ---- heavyweight phase-3 constants: loaded later (see below) --
            wo_sb = cpool.tile([128, 8, D], BF16)
            gam_sb = cpool.tile([128, D], F32)
            bet_sb = cpool.tile([128, D], F32)
            resp_sb = cpool.tile([128, 4, D], F32)  # all residual chunks

            # ---- A2A buffers ----
            # Row ownership is interleaved at 128-row granularity across BOTH
            # batches: core j owns rows [512*(j//2)+128*(j%2), +128) of each
            # batch (even-tile set, parity 0) plus the same +256 (odd set).
            # Every A2A slot then carries real data -- no batch-dup zeros, no
            # receive-side select -- at half the previous payload.
            a2a_in = [
                dram.tile([NC, DVC, 128], BF16, name=f"a2a_in{i}") for i in range(2)
            ]
            a2a_out = [
                dram.tile([NC, DVC, 128], BF16, name=f"a2a_out{i}") for i in range(2)
            ]

            def proj_kq(w_sb, xt_d, b_sb, o_sb, c0):
                # project 256 source columns [c0, c0+256) into o_sb (K^T/Q^T)
                xts = xtpool.tile([128, 8, 256], BF16, tag="xt")
                nc.sync.dma_start(xts[:], xt_d[:, c0 // 256, :, :])
                for m in range(2):
                    ps = psm.tile([128, 512], F32, tag="m")
                    for cc in range(8):
                        nc.tensor.matmul(
                            ps[:, 0:256],
                            lhsT=w_sb[:, cc, 128 * m:128 * m + 128],
                            rhs=xts[:, cc, :],
                            start=(cc == 0),
                            stop=(cc == 7),
                        )
                    nc.vector.tensor_scalar(
                        out=o_sb[:, m, c0:c0 + 256],
                        in0=ps[:, 0:256],
                        scalar1=b_sb[:, m:m + 1],
                        scalar2=None,
                        op0=ADD,
                    )

            def proj_kq512(w_sb, xt_d, b_sb, o_sb, c0):
                # project 512 source columns [c0, c0+512) in N=512 matmuls:
                # half the instruction count of two 256-col chunks, better
                # LDWEIGHTS amortization.
                xts = xtpool5.tile([128, 2, 8, 256], BF16, tag="xt5")
                nc.sync.dma_start(xts[:], xt_d[:, c0 // 256:c0 // 256 + 2, :, :])
                for m in range(2):
                    ps = psm.tile([128, 512], F32, tag="m")
                    for cc in range(8):
                        nc.tensor.matmul(
                            ps[:],
                            lhsT=w_sb[:, cc, 128 * m:128 * m + 128],
                            rhs=xts[:, :, cc, :],
                            start=(cc == 0),
                            stop=(cc == 7),
                        )
                    nc.vector.tensor_scalar(
                        out=o_sb[:, m, c0:c0 + 512],
                        in0=ps[:],
                        scalar1=b_sb[:, m:m + 1],
                        scalar2=None,
                        op0=ADD,
                    )

            def proj_v(c0):
                # project V for k rows [c0, c0+256) (two 128-blocks)
                xvs = xtpool.tile([128, 8, 256], BF16, tag="xt")
                nc.sync.dma_start(xvs[:], xtv[:, c0 // 256, :, :])
                for r in range(2):
                    rc = c0 // 128 + r
                    ps = psm.tile([128, 512], F32, tag="m")
                    for cc in range(8):
                        nc.tensor.matmul(
                            ps[:, 0:DVC],
                            lhsT=xvs[:, cc, 128 * r:128 * r + 128],
                            rhs=wv_sb[:, cc, :],
                            start=(cc == 0),
                            stop=(cc == 7),
                        )
                    v_slot = V_sb[:, rc, :].rearrange("p (h x) -> p h x", x=ND + 4)[
                        :, :, 0:ND
                    ]
                    nc.vector.tensor_tensor(
                        out=v_slot,
                        in0=ps[:, 0:DVC].rearrange("p (h x) -> p h x", x=ND),
                        in1=bv_sb.rearrange("p (h x) -> p h x", x=ND),
                        op=ADD,
                    )

            # pt layout per (hc, jp): [128 kpos, hp(2) x u(2) x q(256)].
            # PV for head parity hp: rhs = pt[:, hp, :, :] (contraction over
            # the jp's two 128-blocks via DoubleRow fp8).
            # PSUM banking: start=True clears has_written for the WHOLE bank,
            # so two accumulation chains must never interleave inside one
            # bank. Bank = ctxps_pair[hp]; within it the hc=0 chain fully
            # precedes the hc=1 chain (cols 256*hc) -- sequential per bank.
            def emit_pv(unit):
                hc, jp, pt, ctxps_pair, t = unit
                ptv = pt.rearrange("p (f u q) -> p f u q", u=2, q=256)
                for hp in range(2):
                    h = 2 * hc + hp
                    nc.tensor.matmul(
                        ctxps_pair[hp][0:ND + 4, 256 * hc:256 * hc + 256],
                        lhsT=V_sb[:, 2 * jp:2 * jp + 2, (ND + 4) * h:(ND + 4) * (h + 1)],
                        rhs=ptv[:, hp, :, :],
                        start=(jp == 0),
                        stop=(jp == t),
                        perf_mode=mybir.MatmulPerfMode.DoubleRow,
                        skip_group_check=True,
                    )

            # Deferred per-tile epilogue, run during iteration t+1 so the PE
            # never waits on the reciprocal chain: denominator reciprocal on
            # DVE straight out of PSUM, partition-broadcast on GPSIMD, then
            # the normalize-divides and the ship DMAs of tile t.
            def finish_tile(pend):
                t, ctxps_pair = pend
                dcp = dnpool.tile([1, 1024], F32, tag="dcp")
                dn0 = dnpool.tile([1, 1024], F32, tag="dn0")
                rcp = dnpool.tile([64, 1024], F32, tag="rcp")
                for pi in range(2):
                    # DVE copies PSUM->SBUF (keeps the exp-saturated ACT
                    # queue free; custom-DVE recip cannot read PSUM itself)
                    nc.vector.tensor_copy(
                        dcp[0:1, 512 * pi:512 * pi + 512],
                        ctxps_pair[pi][64:65, 0:512],
                    )
                    nc.vector.reciprocal_approx_fast(
                        out=dn0[0:1, 512 * pi:512 * pi + 512],
                        in_=dcp[0:1, 512 * pi:512 * pi + 512],
                    )
                nc.gpsimd.partition_broadcast(rcp[:], dn0[0:1, :], channels=64)
                for h in range(HPC):
                    hp = h % 2
                    hc = h // 2
                    po = 64 * hp
                    co = 256 * hc
                    nc.vector.tensor_tensor(
                        out=ctx_sb[po:po + 64, hc, QT * t:QT * t + QT],
                        in0=ctxps_pair[hp][0:64, co:co + 256],
                        in1=rcp[:, 512 * hp + co:512 * hp + co + 256],
                        op=MUL,
                    )
                ha = t % 2
                for hh in range(2):
                    dest = (t - ha) + hh
                    nc.sync.dma_start(
                        a2a_in[ha][dest].rearrange("(m p) q -> p m q", p=128),
                        ctx_sb[:, :, QT * t + 128 * hh:QT * t + 128 * hh + 128],
                    )
                if t == 6:
                    nc.gpsimd.collective_compute(
                        "AllToAll",
                        mybir.AluOpType.bypass,
                        replica_groups=groups,
                        ins=[a2a_in[0].opt()],
                        outs=[a2a_out[0].opt()],
                    )

            # ================= main loop =================
            # Attention is emitted as a stream of (hc, jp) units; each unit's
            # PV matmuls are issued one unit later (pending_pv) so the PE's
            # strict-FIFO queue is never parked behind an un-finished Exp.
            pending = None
            pending_pv = None

            def flush_pv():
                nonlocal pending_pv
                if pending_pv is not None:
                    emit_pv(pending_pv)
                    pending_pv = None

            for i, t in enumerate(ITERS):
                # ---- streamed projections for this iteration ----
                blocks = KV_SCHED[t]
                if len(blocks) == 4:
                    proj_kq512(wk_sb, xtk, bk_sb, KT_sb, blocks[0] * 128)
                else:
                    for p0 in range(0, len(blocks), 2):
                        proj_kq(wk_sb, xtk, bk_sb, KT_sb, blocks[p0] * 128)
                proj_kq(wq_sb, xtq, bq_sb, QT_sb, QT * t)
                flush_pv()
                for p0 in range(0, len(blocks), 2):
                    proj_v(blocks[p0] * 128)
                if pending is not None:
                    finish_tile(pending)
                    pending = None
                if i == 4:
                    # phase-3 constants: load mid-flight, off the hot window
                    nc.sync.dma_start(wo_sb[:], woT)
                    nc.sync.dma_start(gam_sb[:], gam_bc)
                    nc.sync.dma_start(bet_sb[:], bet_bc)
                    # prefetch the residual rows now; two of these otherwise
                    # load on the post-collective tail
                    nc.sync.dma_start(
                        resp_sb[:], resid.rearrange("(r p) n -> p r n", p=128)
                    )

                # ---- attention for q-tile t ----
                ctxps_pair = []
                for _pi in range(2):
                    cpt = psc.tile([128, 512], F32, tag="c")
                    ctxps_pair.append(cpt)
                for hc in range(2):
                    for jp in range(t + 1):
                        sps = pss.tile([128, 1024], F32, tag="s")
                        # two heads' scores issued adjacently: different PE
                        # row groups -> they run concurrently.
                        for u in range(2):
                            for hp in range(2):
                                po = 64 * hp
                                nc.tensor.matmul(
                                    sps[:, 512 * hp + 256 * u:512 * hp + 256 * u + 256],
                                    lhsT=KT_sb[
                                        po:po + 64,
                                        hc,
                                        128 * (2 * jp + u):128 * (2 * jp + u) + 128,
                                    ],
                                    rhs=QT_sb[po:po + 64, hc, QT * t:QT * t + QT],
                                    start=True,
                                    stop=True,
                                )
                        pt = ptpool.tile([128, 1024], FP8, tag="pt")
                        # bias -2 keeps exp() under fp8e4 max; it cancels
                        # in softmax (the ones-column denominator sums the
                        # same fp8 values).
                        nc.scalar.activation(
                            pt[:], sps[:], AF.Exp, scale=SCALE, bias=nbias_sb
                        )
                        if jp == t:
                            nc.vector.tensor_tensor(
                                out=pt[:], in0=pt[:], in1=mo8_sb, op=MUL
                            )
                        flush_pv()
                        pending_pv = (hc, jp, pt, ctxps_pair, t)
                pending = (t, ctxps_pair)

            # tile 7: last PV + epilogue + collective #1 dispatch FIRST, so
            # the collective's flight overlaps the ha=0 output projection.
            flush_pv()
            finish_tile(pending)
            nc.gpsimd.collective_compute(
                "AllToAll",
                mybir.AluOpType.bypass,
                replica_groups=groups,
                ins=[a2a_in[1].opt()],
                outs=[a2a_out[1].opt()],
            )

            # ---- phase 3: gather + output projection + residual + LN ----
            half = 1.5
            # both parities' gathers up-front on the (now idle) ACT queue so
            # the ha=1 loads fire the moment collective #1 lands instead of
            # queuing behind ha=0's writebacks
            gaths = []
            for ha in range(2):
                gath = gathpool.tile([128, 2, 8, 128], BF16, tag=f"gath{ha}")
                for bb in range(2):
                    nc.scalar.dma_start(
                        gath[:, bb, :, :].rearrange("p (s m) q -> p s m q", m=2),
                        a2a_out[ha][4 * bb:4 * bb + 4].rearrange(
                            "s (m p) q -> p s m q", p=128
                        ),
                    )
                gaths.append(gath)
            for ha in range(2):
                gath = gaths[ha]
                for rc in range(2):
                    R = 2 * ha + rc  # local 128-row chunk index (batch rc)
                    y_sb = lnpool.tile([128, D], F32, tag="y")
                    for n in range(2):
                        ps = psm.tile([128, 512], F32, tag="m")
                        for d2 in range(8):
                            nc.tensor.matmul(
                                ps[:],
                                lhsT=gath[:, rc, d2, :],
                                rhs=wo_sb[:, d2, 512 * n:512 * n + 512],
                                start=(d2 == 0),
                                stop=(d2 == 7),
                            )
                        nc.vector.tensor_tensor(
                            out=y_sb[:, 512 * n:512 * n + 512],
                            in0=ps[:],
                            in1=resp_sb[:, R, 512 * n:512 * n + 512],
                            op=ADD,
                        )
                    # LayerNorm over D: bn_stats mean/var + DVE rsqrt bit-trick
                    st = lnpool.tile([128, 16], F32, tag="st")
                    sti = lnpool.tile([128, 2], I32, tag="sti")
                    nc.vector.bn_stats(st[:, 0:6], y_sb[:, 0:512])
                    nc.vector.bn_stats(st[:, 6:12], y_sb[:, 512:1024])
                    nc.vector.bn_aggr(st[:, 12:14], st[:, 0:12])
                    mu = st[:, 12:13]
                    # v = var + eps; y0 = bitcast(0x5f3759df - (v_int >> 1))
                    nc.vector.tensor_tensor(
                        out=st[:, 14:15], in0=st[:, 13:14], in1=eps_sb, op=ADD
                    )
                    v = st[:, 14:15]
                    nc.vector.tensor_scalar(
                        out=sti[:, 0:1], in0=v.bitcast(I32), scalar1=1,
                        scalar2=None, op0=SHR,
                    )
                    nc.vector.tensor_scalar(
                        out=sti[:, 1:2], in0=sti[:, 0:1], scalar1=-1,
                        scalar2=0x5F3759DF, op0=MUL, op1=ADD,
                    )
                    y0 = sti[:, 1:2].bitcast(F32)
                    # h2 = -0.5 v ; two Newton steps: y <- y*(1.5 + h2*y*y)
                    nc.vector.tensor_scalar(
                        out=st[:, 15:16], in0=v, scalar1=-0.5, scalar2=None, op0=MUL
                    )
                    h2 = st[:, 15:16]
                    nc.vector.tensor_tensor(out=st[:, 0:1], in0=y0, in1=y0, op=MUL)
                    nc.vector.tensor_scalar(
                        out=st[:, 1:2], in0=st[:, 0:1], scalar1=h2, scalar2=half,
                        op0=MUL, op1=ADD,
                    )
                    nc.vector.tensor_tensor(out=st[:, 2:3], in0=y0, in1=st[:, 1:2], op=MUL)
                    nc.vector.tensor_tensor(
                        out=st[:, 3:4], in0=st[:, 2:3], in1=st[:, 2:3], op=MUL
                    )
                    nc.vector.tensor_scalar(
                        out=st[:, 4:5], in0=st[:, 3:4], scalar1=h2, scalar2=half,
                        op0=MUL, op1=ADD,
                    )
                    nc.vector.tensor_tensor(out=st[:, 5:6], in0=st[:, 2:3], in1=st[:, 4:5], op=MUL)
                    rstd = st[:, 5:6]
                    # yc = (y - mu) * rstd ; out = yc*gamma + beta
                    yc = lnpool.tile([128, D], F32, tag="yc")
                    nc.vector.scalar_tensor_tensor(
                        out=yc[:], in0=y_sb[:], scalar=mu, in1=gam_sb[:],
                        op0=SUB, op1=MUL,
                    )
                    nc.vector.scalar_tensor_tensor(
                        out=yc[:], in0=yc[:], scalar=rstd, in1=bet_sb[:],
                        op0=MUL, op1=ADD,
                    )
                    nc.sync.dma_start(out_d[128 * R:128 * R + 128, :], yc[:])

    nc.compile()
    return nc


def _prep_inputs(x_q, x_k, x_v, mask, Wq, bq, Wk, bk, Wv, bv, Wo, bo, gamma, beta):
    import ml_dtypes

    f = np.float32
    bf = ml_dtypes.bfloat16
    maskA = np.zeros((KB, QT), f)
    maskB = np.zeros((KB, QT), f)
    for i in range(KB):
        maskA[i, i:] = 1.0
        if i + 128 < QT:
            maskB[i, i + 128:] = 1.0
    mo1 = np.concatenate([maskA, maskB], axis=1)
    mo8 = np.concatenate([mo1, mo1], axis=1).astype(ml_dtypes.float8_e4m3)

    def shuf_w(w):  # [D, n] -> [128, 8, n] with row p,c = w[c*128+p]
        return np.ascontiguousarray(
            w.reshape(8, 128, w.shape[1]).transpose(1, 0, 2).astype(bf)
        )

    def shuf_x(x):  # [D, S] -> [128, 8(qtile), 8(cblk), 256]
        # [p, T, c, j] = x[c*128+p, 256*T+j]
        x4 = x.reshape(8, 128, 8, 256)  # [c, p, T, j]
        return np.ascontiguousarray(x4.transpose(1, 2, 0, 3).astype(bf))

    in_maps = []
    for c in range(NC):
        b, g = c // 4, c % 4
        dv = slice(DVC * g, DVC * (g + 1))
        # interleaved cross-batch row ownership (see A2A comment in _build)
        re = 512 * (c // 2) + 128 * (c % 2)
        ro = re + 256
        smallc = np.zeros((128, 288), f)
        smallc[:, 0:2] = bq[dv].astype(f).reshape(2, 128).T
        smallc[:, 2:4] = bk[dv].astype(f).reshape(2, 128).T
        smallc[:, 4] = 1.0 - b
        smallc[:, 5] = float(b)
        smallc[:, 12] = EPS
        smallc[:, 13] = -2.0
        smallc[:, 16:16 + DVC] = np.broadcast_to(bv[dv].astype(f), (128, DVC))
        in_maps.append(
            {
                "xtq": shuf_x(x_q[b].T),
                "xtk": shuf_x(x_k[b].T),
                "xtv": shuf_x(x_v[b].T),
                "wqT": shuf_w(Wq[dv, :].T),
                "wkT": shuf_w(Wk[dv, :].T),
                "wvT": shuf_w(Wv[dv, :].T),
                "woT": shuf_w(Wo.T),
                "smallc": smallc,
                "gam_bc": np.broadcast_to(gamma.astype(f), (128, D)).copy(),
                "bet_bc": np.broadcast_to(beta.astype(f), (128, D)).copy(),
                "resid": np.ascontiguousarray(
                    np.concatenate(
                        [
                            x_q[0, re:re + 128, :],
                            x_q[1, re:re + 128, :],
                            x_q[0, ro:ro + 128, :],
                            x_q[1, ro:ro + 128, :],
                        ]
                    ).astype(f)
                    + bo.astype(f)
                ),
                "mo8": mo8,
                "ones_r": np.ones((1, 64), f),
            }
        )
    return in_maps


def kernel(x_q, x_k, x_v, mask, Wq, bq, Wk, bk, Wv, bv, Wo, bo, gamma, beta):
    _install_ntff_shim()
    from concourse.bass_utils import run_bass_kernel_spmd

    x_q, x_k, x_v = np.asarray(x_q), np.asarray(x_k), np.asarray(x_v)
    mask = np.asarray(mask)
    # this kernel implements causal attention structurally; verify the mask
    causal = np.tril(np.ones((S, S), mask.dtype))
    assert np.array_equal(mask.reshape(S, S), causal), "kernel specialized for causal mask"

    if "nc" not in _cache:
        _cache["nc"] = _build()
    nc = _cache["nc"]

    in_maps = _prep_inputs(
        x_q, x_k, x_v, mask,
        np.asarray(Wq), np.asarray(bq), np.asarray(Wk), np.asarray(bk),
        np.asarray(Wv), np.asarray(bv), np.asarray(Wo), np.asarray(bo),
        np.asarray(gamma), np.asarray(beta),
    )
    res = run_bass_kernel_spmd(nc, in_maps, list(range(NC)))
    _cache["last_results"] = res

    out = np.empty((B, S, D), np.float32)
    for c in range(NC):
        re = 512 * (c // 2) + 128 * (c % 2)
        ro = re + 256
        r = res.results[c]["out"]
        out[0, re:re + 128, :] = r[0:128]
        out[1, re:re + 128, :] = r[128:256]
        out[0, ro:ro + 128, :] = r[256:384]
        out[1, ro:ro + 128, :] = r[384:512]
    return out


# revision 52
# speedup vs baseline: 1.0258x; 1.0258x over previous
# Trainium2 Bass kernel for nn_MultiHeadAttention_87024627352037.
#
# Full module: y = LayerNorm(x_q + (softmax(mask(QK^T/sqrt(nd))) V) Wo^T + bo)
# with Q/K/V projections of x_q/x_k/x_v. Shapes: B=2, S=2048, D=1024, H=16.
#
# Sharding (8 cores): core c = (batch b=c//4, head-quad g=c%4).
# Each core projects Q/K/V for its 4 heads (dv=256) over its batch and runs
# causal attention in a fully transposed layout (scoresT = K_T^T Q_T, no
# max-subtraction -- scores are O(1); softmax denominator via a ones-column
# in the PV matmul). Projections are streamed and interleaved with the
# attention q-tiles (processed 0,2,4,6,1,3,5,7) so the PE ramps early and
# stays busy.
#
# Scores matmuls for the two heads of a partition pair (rows 0-63 / 64-127)
# are issued adjacently so the PE runs them CONCURRENTLY in different row
# groups (tile_position auto-derived from base partitions); one Exp covers
# both heads' PSUM banks. The softmax denominator reciprocal runs on DVE
# straight out of PSUM and is broadcast across partitions by the (otherwise
# idle) GPSIMD engine -- the ACT engine runs *only* Exp and the PE runs only
# real matmuls. PV matmuls are software-pipelined one step behind the
# scores/exp so the strict-FIFO PE queue never head-of-line blocks on ACT.
# A per-batch AllToAll (groups of 8) re-shards ctx from head-sharding to
# row-sharding; each core computes output projection + residual + LayerNorm
# for its 512 rows. The host only slices, transposes, and concatenates
# numpy arrays.
import os
import sys
import types

import numpy as np

B, S, D, H = 2, 2048, 1024, 16
ND = D // H          # 64
NC = 8               # cores
HPC = H // 4         # 4 heads per core
DVC = HPC * ND       # 256 dv per core
QT = 256             # q tile
NQT = S // QT        # 8 q tiles
KB = 128             # k block
EPS = 1e-5
SCALE = 1.0 / np.sqrt(ND)

# iteration order: even tiles first so the even-parity AllToAll can fire at
# ~44% of the attention work and overlap the odd-tile compute.
ITERS = (0, 2, 4, 6, 1, 3, 5, 7)
# K/V 128-blocks projected at each iteration (front-loaded so tile t always
# has K/V blocks 0..2t+1 available).
KV_SCHED = {0: (0, 1), 2: (2, 3, 4, 5), 4: (6, 7, 8, 9), 6: (10, 11, 12, 13),
            1: (14, 15), 3: (), 5: (), 7: ()}

_cache = {}


def _install_ntff_shim():
    # antenv.axon_hooks is absent in this image; register the NTFF profile
    # hook so trace=True can capture HW exec time (harmless if unused).
    if "antenv.axon_hooks" in sys.modules:
        return
    mod = types.ModuleType("antenv.axon_hooks")
    mod._hook = None
    mod.set_axon_ntff_profile_hook = lambda h: setattr(mod, "_hook", h)
    mod.get_axon_ntff_profile_hook = lambda: mod._hook
    sys.modules["antenv.axon_hooks"] = mod
    try:
        import antenv

        antenv.axon_hooks = mod
        from trn_agent_boot.trn_boot import _ntff_profile_via_ctypes

        mod._hook = _ntff_profile_via_ctypes("/opt/axon/libaxon_pjrt.so")
    except Exception:
        pass


def _build():
    import concourse.bass as bass
    import concourse.mybir as mybir
    import concourse.tile as tile
    from concourse import bacc

    F32 = mybir.dt.float32
    F32R = mybir.dt.float32r
    BF16 = mybir.dt.bfloat16
    FP8 = mybir.dt.float8e4
    I32 = mybir.dt.int32
    ADD = mybir.AluOpType.add
    MUL = mybir.AluOpType.mult
    SUB = mybir.AluOpType.subtract
    SHR = mybir.AluOpType.logical_shift_right
    AF = mybir.ActivationFunctionType

    nc = bacc.Bacc("TRN2", target_bir_lowering=False, debug=False, num_devices=NC)

    def din(name, shape, dt=BF16):
        return nc.dram_tensor(name, shape, dt, kind="ExternalInput").ap()

    # host pre-shuffled layouts: partition-major so every DMA is contiguous
    # 4KB+ per partition (8x fewer descriptors than (c p)->p c rearranges).
    xtq = din("xtq", [128, 8, 8, 256])   # [p, qtile, cblk, col]
    xtk = din("xtk", [128, 8, 8, 256])
    xtv = din("xtv", [128, 8, 8, 256])
    wqT = din("wqT", [128, 8, DVC])
    wkT = din("wkT", [128, 8, DVC])
    wvT = din("wvT", [128, 8, DVC])
    woT = din("woT", [128, 8, D])
    smallc = din("smallc", [128, 288], F32)   # bq2|bk2|eps|pad|bv4x64(@16)
    gam_bc = din("gam_bc", [128, D], F32)
    bet_bc = din("bet_bc", [128, D], F32)
    resid = din("resid", [512, D], F32)       # x_q rows + bo (host pre-added)
    mo8_in = din("mo8", [128, 4 * QT], mybir.dt.float8e4)  # diag mask x2 heads
    ones_r = din("ones_r", [1, 64], F32R)
    out_d = nc.dram_tensor("out", [512, D], F32, kind="ExternalOutput").ap()

    groups = [list(range(NC))]

    with nc.allow_low_precision(reason="f32r/bf16 matmul operand chain"), tile.TileContext(
        nc
    ) as tc:
        with (
            tc.tile_pool(name="const", bufs=1) as cpool,
            tc.tile_pool(name="res", bufs=1) as rpool,
            tc.tile_pool(name="xt", bufs=8) as xtpool,
            tc.tile_pool(name="xt5", bufs=2) as xtpool5,
            tc.tile_pool(name="pt", bufs=4) as ptpool,
            tc.tile_pool(name="dn", bufs=3) as dnpool,
            tc.tile_pool(name="gath", bufs=1) as gathpool,
            tc.tile_pool(name="ln", bufs=2) as lnpool,
            tc.tile_pool(name="ps_s", bufs=2, space="PSUM") as pss,
            tc.tile_pool(name="ps_ctx", bufs=2, space="PSUM") as psc,
            tc.tile_pool(name="ps_m", bufs=2, space="PSUM") as psm,
            tc.tile_pool(name="dram", bufs=1, space="DRAM") as dram,
        ):
            # ---- small constants + projection weights (needed first) ----
            smallc_sb = cpool.tile([128, 288], F32)
            mo8_sb = cpool.tile([128, 4 * QT], FP8)
            wq_sb = cpool.tile([128, 8, DVC], BF16)
            wk_sb = cpool.tile([128, 8, DVC], BF16)
            wv_sb = cpool.tile([128, 8, DVC], BF16)
            # spread startup loads across engine queues so issue overlaps
            nc.sync.dma_start(wk_sb[:], wkT)
            nc.scalar.dma_start(wq_sb[:], wqT)
            nc.gpsimd.dma_start(wv_sb[:], wvT)
            nc.sync.dma_start(smallc_sb[:], smallc)
            ones_sb = cpool.tile([1, 64], F32R)
            nc.sync.dma_start(ones_sb[:], ones_r)
            nc.sync.dma_start(mo8_sb[:], mo8_in)
            bq_sb = smallc_sb[:, 0:2]
            bk_sb = smallc_sb[:, 2:4]
            eps_sb = smallc_sb[:, 12:13]
            nbias_sb = smallc_sb[:, 13:14]  # -2.0 exp bias
            bv_sb = smallc_sb[:, 16:16 + DVC]   # bv broadcast (no ones col)

            # ---- resident activation tensors ----
            QT_sb = rpool.tile([128, 2, S], BF16)   # q^T: [dd(2x128), q]
            KT_sb = rpool.tile([128, 2, S], BF16)   # k^T: [dd(2x128), kpos]
            V_sb = rpool.tile([128, S // 128, HPC * (ND + 4)], FP8)
            ctx_sb = rpool.tile([128, 2, S], BF16)  # ctx^T: [dv(2x128), q]
            # ones columns of the V slots (denominator trick), set once;
            # slots are 68 wide (16B-aligned strides for dual-fp8 ldweights):
            # 64 data cols, a ones col, 3 zero pad cols. Data cols are fully
            # written by proj_v before use -- only pad+ones need memset.
            v4 = V_sb[:].rearrange("p c (h x) -> p c h x", x=ND + 4)
            nc.gpsimd.memset(v4[:, :, :, ND:ND + 4], 0.0)
            nc.gpsimd.memset(v4[:, :, :, ND:ND + 1], 1.0)

            # ---- heavyweight phase-3 constants: loaded later (see below) --
            wo_sb = cpool.tile([128, 8, D], BF16)
            gam_sb = cpool.tile([128, D], F32)
            bet_sb = cpool.tile([128, D], F32)
            resp_sb = cpool.tile([128, 4, D], F32)  # all residual chunks

            # ---- A2A buffers ----
            # Row ownership is interleaved at 128-row granularity across BOTH
            # batches: core j owns rows [512*(j//2)+128*(j%2), +128) of each
            # batch (even-tile set, parity 0) plus the same +256 (odd set).
            # Every A2A slot then carries real data -- no batch-dup zeros, no
            # receive-side select -- at half the previous payload.
            a2a_in = [
                dram.tile([NC, DVC, 128], BF16, name=f"a2a_in{i}") for i in range(2)
            ]
            a2a_out = [
                dram.tile([NC, DVC, 128], BF16, name=f"a2a_out{i}") for i in range(2)
            ]

            def proj_kq(w_sb, xt_d, b_sb, o_sb, c0):
                # project 256 source columns [c0, c0+256) into o_sb (K^T/Q^T)
                xts = xtpool.tile([128, 8, 256], BF16, tag="xt")
                nc.sync.dma_start(xts[:], xt_d[:, c0 // 256, :, :])
                for m in range(2):
                    ps = psm.tile([128, 512], F32, tag="m")
                    for cc in range(8):
                        nc.tensor.matmul(
                            ps[:, 0:256],
                            lhsT=w_sb[:, cc, 128 * m:128 * m + 128],
                            rhs=xts[:, cc, :],
                            start=(cc == 0),
                            stop=(cc == 7),
                        )
                    nc.vector.tensor_scalar(
                        out=o_sb[:, m, c0:c0 + 256],
                        in0=ps[:, 0:256],
                        scalar1=b_sb[:, m:m + 1],
                        scalar2=None,
                        op0=ADD,
                    )

            def proj_kq512(w_sb, xt_d, b_sb, o_sb, c0):
                # project 512 source columns [c0, c0+512) in N=512 matmuls:
                # half the instruction count of two 256-col chunks, better
                # LDWEIGHTS amortization.
                xts = xtpool5.tile([128, 2, 8, 256], BF16, tag="xt5")
                nc.sync.dma_start(xts[:], xt_d[:, c0 // 256:c0 // 256 + 2, :, :])
                for m in range(2):
                    ps = psm.tile([128, 512], F32, tag="m")
                    for cc in range(8):
                        nc.tensor.matmul(
                            ps[:],
                            lhsT=w_sb[:, cc, 128 * m:128 * m + 128],
                            rhs=xts[:, :, cc, :],
                            start=(cc == 0),
                            stop=(cc == 7),
                        )
                    nc.vector.tensor_scalar(
                        out=o_sb[:, m, c0:c0 + 512],
                        in0=ps[:],
                        scalar1=b_sb[:, m:m + 1],
                        scalar2=None,
                        op0=ADD,
                    )

            def proj_v(c0):
                # project V for k rows [c0, c0+256) (two 128-blocks)
                xvs = xtpool.tile([128, 8, 256], BF16, tag="xt")
                nc.sync.dma_start(xvs[:], xtv[:, c0 // 256, :, :])
                for r in range(2):
                    rc = c0 // 128 + r
                    ps = psm.tile([128, 512], F32, tag="m")
                    for cc in range(8):
                        nc.tensor.matmul(
                            ps[:, 0:DVC],
                            lhsT=xvs[:, cc, 128 * r:128 * r + 128],
                            rhs=wv_sb[:, cc, :],
                            start=(cc == 0),
                            stop=(cc == 7),
                        )
                    v_slot = V_sb[:, rc, :].rearrange("p (h x) -> p h x", x=ND + 4)[
                        :, :, 0:ND
                    ]
                    nc.vector.tensor_tensor(
                        out=v_slot,
                        in0=ps[:, 0:DVC].rearrange("p (h x) -> p h x", x=ND),
                        in1=bv_sb.rearrange("p (h x) -> p h x", x=ND),
                        op=ADD,
                    )

            # pt layout per (hc, jp): [128 kpos, hp(2) x u(2) x q(256)].
            # PV for head parity hp: rhs = pt[:, hp, :, :] (contraction over
            # the jp's two 128-blocks via DoubleRow fp8).
            # PSUM banking: start=True clears has_written for the WHOLE bank,
            # so two accumulation chains must never interleave inside one
            # bank. Bank = ctxps_pair[hp]; within it the hc=0 chain fully
            # precedes the hc=1 chain (cols 256*hc) -- sequential per bank.
            def emit_pv(unit):
                hc, jp, pt, ctxps_pair, t = unit
                ptv = pt.rearrange("p (f u q) -> p f u q", u=2, q=256)
                for hp in range(2):
                    h = 2 * hc + hp
                    nc.tensor.matmul(
                        ctxps_pair[hp][0:ND + 4, 256 * hc:256 * hc + 256],
                        lhsT=V_sb[:, 2 * jp:2 * jp + 2, (ND + 4) * h:(ND + 4) * (h + 1)],
                        rhs=ptv[:, hp, :, :],
                        start=(jp == 0),
                        stop=(jp == t),
                        perf_mode=mybir.MatmulPerfMode.DoubleRow,
                        skip_group_check=True,
                    )

            # Deferred per-tile epilogue, run during iteration t+1 so the PE
            # never waits on the reciprocal chain: denominator reciprocal on
            # DVE straight out of PSUM, partition-broadcast on GPSIMD, then
            # the normalize-divides and the ship DMAs of tile t.
            def finish_tile(pend):
                t, ctxps_pair = pend
                dcp = dnpool.tile([1, 1024], F32, tag="dcp")
                dn0 = dnpool.tile([1, 1024], F32, tag="dn0")
                rcp = dnpool.tile([64, 1024], F32, tag="rcp")
                for pi in range(2):
                    # ACT copies PSUM->SBUF (custom-DVE ops cannot read PSUM)
                    nc.scalar.activation(
                        dcp[0:1, 512 * pi:512 * pi + 512],
                        ctxps_pair[pi][64:65, 0:512],
                        AF.Copy,
                    )
                    nc.vector.reciprocal_approx_fast(
                        out=dn0[0:1, 512 * pi:512 * pi + 512],
                        in_=dcp[0:1, 512 * pi:512 * pi + 512],
                    )
                nc.gpsimd.partition_broadcast(rcp[:], dn0[0:1, :], channels=64)
                for h in range(HPC):
                    hp = h % 2
                    hc = h // 2
                    po = 64 * hp
                    co = 256 * hc
                    nc.vector.tensor_tensor(
                        out=ctx_sb[po:po + 64, hc, QT * t:QT * t + QT],
                        in0=ctxps_pair[hp][0:64, co:co + 256],
                        in1=rcp[:, 512 * hp + co:512 * hp + co + 256],
                        op=MUL,
                    )
                ha = t % 2
                for hh in range(2):
                    dest = (t - ha) + hh
                    nc.sync.dma_start(
                        a2a_in[ha][dest].rearrange("(m p) q -> p m q", p=128),
                        ctx_sb[:, :, QT * t + 128 * hh:QT * t + 128 * hh + 128],
                    )
                if t == 6:
                    nc.gpsimd.collective_compute(
                        "AllToAll",
                        mybir.AluOpType.bypass,
                        replica_groups=groups,
                        ins=[a2a_in[0].opt()],
                        outs=[a2a_out[0].opt()],
                    )

            # ================= main loop =================
            # Attention is emitted as a stream of (hc, jp) units; each unit's
            # PV matmuls are issued one unit later (pending_pv) so the PE's
            # strict-FIFO queue is never parked behind an un-finished Exp.
            pending = None
            pending_pv = None

            def flush_pv():
                nonlocal pending_pv
                if pending_pv is not None:
                    emit_pv(pending_pv)
                    pending_pv = None

            for i, t in enumerate(ITERS):
                # ---- streamed projections for this iteration ----
                blocks = KV_SCHED[t]
                if len(blocks) == 4:
                    proj_kq512(wk_sb, xtk, bk_sb, KT_sb, blocks[0] * 128)
                else:
                    for p0 in range(0, len(blocks), 2):
                        proj_kq(wk_sb, xtk, bk_sb, KT_sb, blocks[p0] * 128)
                proj_kq(wq_sb, xtq, bq_sb, QT_sb, QT * t)
                flush_pv()
                for p0 in range(0, len(blocks), 2):
                    proj_v(blocks[p0] * 128)
                if pending is not None:
                    finish_tile(pending)
                    pending = None
                if i == 4:
                    # phase-3 constants: load mid-flight, off the hot window
                    nc.sync.dma_start(wo_sb[:], woT)
                    nc.sync.dma_start(gam_sb[:], gam_bc)
                    nc.sync.dma_start(bet_sb[:], bet_bc)
                    # prefetch the residual rows now; two of these otherwise
                    # load on the post-collective tail
                    nc.sync.dma_start(
                        resp_sb[:], resid.rearrange("(r p) n -> p r n", p=128)
                    )

                # ---- attention for q-tile t ----
                ctxps_pair = []
                for _pi in range(2):
                    cpt = psc.tile([128, 512], F32, tag="c")
                    ctxps_pair.append(cpt)
                for hc in range(2):
                    for jp in range(t + 1):
                        sps = pss.tile([128, 1024], F32, tag="s")
                        # two heads' scores issued adjacently: different PE
                        # row groups -> they run concurrently.
                        for u in range(2):
                            for hp in range(2):
                                po = 64 * hp
                                nc.tensor.matmul(
                                    sps[:, 512 * hp + 256 * u:512 * hp + 256 * u + 256],
                                    lhsT=KT_sb[
                                        po:po + 64,
                                        hc,
                                        128 * (2 * jp + u):128 * (2 * jp + u) + 128,
                                    ],
                                    rhs=QT_sb[po:po + 64, hc, QT * t:QT * t + QT],
                                    start=True,
                                    stop=True,
                                )
                        pt = ptpool.tile([128, 1024], FP8, tag="pt")
                        # bias -2 keeps exp() under fp8e4 max; it cancels
                        # in softmax (the ones-column denominator sums the
                        # same fp8 values).
                        nc.scalar.activation(
                            pt[:], sps[:], AF.Exp, scale=SCALE, bias=nbias_sb
                        )
                        if jp == t:
                            nc.vector.tensor_tensor(
                                out=pt[:], in0=pt[:], in1=mo8_sb, op=MUL
                            )
                        flush_pv()
                        pending_pv = (hc, jp, pt, ctxps_pair, t)
                pending = (t, ctxps_pair)

            # tile 7: last PV + epilogue + collective #1 dispatch FIRST, so
            # the collective's flight overlaps the ha=0 output projection.
            flush_pv()
            finish_tile(pending)
            nc.gpsimd.collective_compute(
                "AllToAll",
                mybir.AluOpType.bypass,
                replica_groups=groups,
                ins=[a2a_in[1].opt()],
                outs=[a2a_out[1].opt()],
            )

            # ---- phase 3: gather + output projection + residual + LN ----
            half = 1.5
            # both parities' gathers up-front on the (now idle) ACT queue so
            # the ha=1 loads fire the moment collective #1 lands instead of
            # queuing behind ha=0's writebacks
            gaths = []
            for ha in range(2):
                gath = gathpool.tile([128, 2, 8, 128], BF16, tag=f"gath{ha}")
                for bb in range(2):
                    nc.scalar.dma_start(
                        gath[:, bb, :, :].rearrange("p (s m) q -> p s m q", m=2),
                        a2a_out[ha][4 * bb:4 * bb + 4].rearrange(
                            "s (m p) q -> p s m q", p=128
                        ),
                    )
                gaths.append(gath)
            for ha in range(2):
                gath = gaths[ha]
                for rc in range(2):
                    R = 2 * ha + rc  # local 128-row chunk index (batch rc)
                    y_sb = lnpool.tile([128, D], F32, tag="y")
                    for n in range(2):
                        ps = psm.tile([128, 512], F32, tag="m")
                        for d2 in range(8):
                            nc.tensor.matmul(
                                ps[:],
                                lhsT=gath[:, rc, d2, :],
                                rhs=wo_sb[:, d2, 512 * n:512 * n + 512],
                                start=(d2 == 0),
                                stop=(d2 == 7),
                            )
                        nc.vector.tensor_tensor(
                            out=y_sb[:, 512 * n:512 * n + 512],
                            in0=ps[:],
                            in1=resp_sb[:, R, 512 * n:512 * n + 512],
                            op=ADD,
                        )
                    # LayerNorm over D: bn_stats mean/var + DVE rsqrt bit-trick
                    st = lnpool.tile([128, 16], F32, tag="st")
                    sti = lnpool.tile([128, 2], I32, tag="sti")
                    nc.vector.bn_stats(st[:, 0:6], y_sb[:, 0:512])
                    nc.vector.bn_stats(st[:, 6:12], y_sb[:, 512:1024])
                    nc.vector.bn_aggr(st[:, 12:14], st[:, 0:12])
                    mu = st[:, 12:13]
                    # v = var + eps; y0 = bitcast(0x5f3759df - (v_int >> 1))
                    nc.vector.tensor_tensor(
                        out=st[:, 14:15], in0=st[:, 13:14], in1=eps_sb, op=ADD
                    )
                    v = st[:, 14:15]
                    nc.vector.tensor_scalar(
                        out=sti[:, 0:1], in0=v.bitcast(I32), scalar1=1,
                        scalar2=None, op0=SHR,
                    )
                    nc.vector.tensor_scalar(
                        out=sti[:, 1:2], in0=sti[:, 0:1], scalar1=-1,
                        scalar2=0x5F3759DF, op0=MUL, op1=ADD,
                    )
                    y0 = sti[:, 1:2].bitcast(F32)
                    # h2 = -0.5 v ; two Newton steps: y <- y*(1.5 + h2*y*y)
                    nc.vector.tensor_scalar(
                        out=st[:, 15:16], in0=v, scalar1=-0.5, scalar2=None, op0=MUL
                    )
                    h2 = st[:, 15:16]
                    nc.vector.tensor_tensor(out=st[:, 0:1], in0=y0, in1=y0, op=MUL)
                    nc.vector.tensor_scalar(
                        out=st[:, 1:2], in0=st[:, 0:1], scalar1=h2, scalar2=half,
                        op0=MUL, op1=ADD,
                    )
                    nc.vector.tensor_tensor(out=st[:, 2:3], in0=y0, in1=st[:, 1:2], op=MUL)
                    nc.vector.tensor_tensor(
                        out=st[:, 3:4], in0=st[:, 2:3], in1=st[:, 2:3], op=MUL
                    )
                    nc.vector.tensor_scalar(
                        out=st[:, 4:5], in0=st[:, 3:4], scalar1=h2, scalar2=half,
                        op0=MUL, op1=ADD,
                    )
                    nc.vector.tensor_tensor(out=st[:, 5:6], in0=st[:, 2:3], in1=st[:, 4:5], op=MUL)
                    rstd = st[:, 5:6]
                    # yc = (y - mu) * rstd ; out = yc*gamma + beta
                    yc = lnpool.tile([128, D], F32, tag="yc")
                    nc.vector.scalar_tensor_tensor(
                        out=yc[:], in0=y_sb[:], scalar=mu, in1=gam_sb[:],
                        op0=SUB, op1=MUL,
                    )
                    nc.vector.scalar_tensor_tensor(
                        out=yc[:], in0=yc[:], scalar=rstd, in1=bet_sb[:],
                        op0=MUL, op1=ADD,
                    )
                    nc.sync.dma_start(out_d[128 * R:128 * R + 128, :], yc[:])

    nc.compile()
    return nc


def _prep_inputs(x_q, x_k, x_v, mask, Wq, bq, Wk, bk, Wv, bv, Wo, bo, gamma, beta):
    import ml_dtypes

    f = np.float32
    bf = ml_dtypes.bfloat16
    maskA = np.zeros((KB, QT), f)
    maskB = np.zeros((KB, QT), f)
    for i in range(KB):
        maskA[i, i:] = 1.0
        if i + 128 < QT:
            maskB[i, i + 128:] = 1.0
    mo1 = np.concatenate([maskA, maskB], axis=1)
    mo8 = np.concatenate([mo1, mo1], axis=1).astype(ml_dtypes.float8_e4m3)

    def shuf_w(w):  # [D, n] -> [128, 8, n] with row p,c = w[c*128+p]
        return np.ascontiguousarray(
            w.reshape(8, 128, w.shape[1]).transpose(1, 0, 2).astype(bf)
        )

    def shuf_x(x):  # [D, S] -> [128, 8(qtile), 8(cblk), 256]
        # [p, T, c, j] = x[c*128+p, 256*T+j]
        x4 = x.reshape(8, 128, 8, 256)  # [c, p, T, j]
        return np.ascontiguousarray(x4.transpose(1, 2, 0, 3).astype(bf))

    in_maps = []
    for c in range(NC):
        b, g = c // 4, c % 4
        dv = slice(DVC * g, DVC * (g + 1))
        # interleaved cross-batch row ownership (see A2A comment in _build)
        re = 512 * (c // 2) + 128 * (c % 2)
        ro = re + 256
        smallc = np.zeros((128, 288), f)
        smallc[:, 0:2] = bq[dv].astype(f).reshape(2, 128).T
        smallc[:, 2:4] = bk[dv].astype(f).reshape(2, 128).T
        smallc[:, 4] = 1.0 - b
        smallc[:, 5] = float(b)
        smallc[:, 12] = EPS
        smallc[:, 13] = -2.0
        smallc[:, 16:16 + DVC] = np.broadcast_to(bv[dv].astype(f), (128, DVC))
        in_maps.append(
            {
                "xtq": shuf_x(x_q[b].T),
                "xtk": shuf_x(x_k[b].T),
                "xtv": shuf_x(x_v[b].T),
                "wqT": shuf_w(Wq[dv, :].T),
                "wkT": shuf_w(Wk[dv, :].T),
                "wvT": shuf_w(Wv[dv, :].T),
                "woT": shuf_w(Wo.T),
                "smallc": smallc,
                "gam_bc": np.broadcast_to(gamma.astype(f), (128, D)).copy(),
                "bet_bc": np.broadcast_to(beta.astype(f), (128, D)).copy(),
                "resid": np.ascontiguousarray(
                    np.concatenate(
                        [
                            x_q[0, re:re + 128, :],
                            x_q[1, re:re + 128, :],
                            x_q[0, ro:ro + 128, :],
                            x_q[1, ro:ro + 128, :],
                        ]
                    ).astype(f)
                    + bo.astype(f)
                ),
                "mo8": mo8,
                "ones_r": np.ones((1, 64), f),
            }
        )
    return in_maps


def kernel(x_q, x_k, x_v, mask, Wq, bq, Wk, bk, Wv, bv, Wo, bo, gamma, beta):
    _install_ntff_shim()
    from concourse.bass_utils import run_bass_kernel_spmd

    x_q, x_k, x_v = np.asarray(x_q), np.asarray(x_k), np.asarray(x_v)
    mask = np.asarray(mask)
    # this kernel implements causal attention structurally; verify the mask
    causal = np.tril(np.ones((S, S), mask.dtype))
    assert np.array_equal(mask.reshape(S, S), causal), "kernel specialized for causal mask"

    if "nc" not in _cache:
        _cache["nc"] = _build()
    nc = _cache["nc"]

    in_maps = _prep_inputs(
        x_q, x_k, x_v, mask,
        np.asarray(Wq), np.asarray(bq), np.asarray(Wk), np.asarray(bk),
        np.asarray(Wv), np.asarray(bv), np.asarray(Wo), np.asarray(bo),
        np.asarray(gamma), np.asarray(beta),
    )
    res = run_bass_kernel_spmd(nc, in_maps, list(range(NC)))
    _cache["last_results"] = res

    out = np.empty((B, S, D), np.float32)
    for c in range(NC):
        re = 512 * (c // 2) + 128 * (c % 2)
        ro = re + 256
        r = res.results[c]["out"]
        out[0, re:re + 128, :] = r[0:128]
        out[1, re:re + 128, :] = r[128:256]
        out[0, ro:ro + 128, :] = r[256:384]
        out[1, ro:ro + 128, :] = r[384:512]
    return out


# revision 56
# speedup vs baseline: 1.0752x; 1.0482x over previous
# Trainium2 Bass kernel for nn_MultiHeadAttention_87024627352037.
#
# Full module: y = LayerNorm(x_q + (softmax(mask(QK^T/sqrt(nd))) V) Wo^T + bo)
# with Q/K/V projections of x_q/x_k/x_v. Shapes: B=2, S=2048, D=1024, H=16.
#
# Sharding (8 cores): core c = (batch b=c//4, head-quad g=c%4).
# Each core projects Q/K/V for its 4 heads (dv=256) over its batch and runs
# causal attention in a fully transposed layout (scoresT = K_T^T Q_T, no
# max-subtraction -- scores are O(1); softmax denominator via a ones-column
# in the PV matmul). Projections are streamed and interleaved with the
# attention q-tiles (processed 0,2,4,6,1,3,5,7) so the PE ramps early and
# stays busy.
#
# Scores matmuls for the two heads of a partition pair (rows 0-63 / 64-127)
# are issued adjacently so the PE runs them CONCURRENTLY in different row
# groups (tile_position auto-derived from base partitions); one Exp covers
# both heads' PSUM banks. The softmax denominator reciprocal runs on DVE
# straight out of PSUM and is broadcast across partitions by the (otherwise
# idle) GPSIMD engine -- the ACT engine runs *only* Exp and the PE runs only
# real matmuls. PV matmuls are software-pipelined one step behind the
# scores/exp so the strict-FIFO PE queue never head-of-line blocks on ACT.
# A per-batch AllToAll (groups of 8) re-shards ctx from head-sharding to
# row-sharding; each core computes output projection + residual + LayerNorm
# for its 512 rows. The host only slices, transposes, and concatenates
# numpy arrays.
import os
import sys
import types

import numpy as np

B, S, D, H = 2, 2048, 1024, 16
ND = D // H          # 64
NC = 8               # cores
HPC = H // 4         # 4 heads per core
DVC = HPC * ND       # 256 dv per core
QT = 256             # q tile
NQT = S // QT        # 8 q tiles
KB = 128             # k block
EPS = 1e-5
SCALE = 1.0 / np.sqrt(ND)

# iteration order: even tiles first so the even-parity AllToAll can fire at
# ~44% of the attention work and overlap the odd-tile compute.
ITERS = (0, 2, 4, 6, 1, 3, 5, 7)
# K/V 128-blocks projected at each iteration (front-loaded so tile t always
# has K/V blocks 0..2t+1 available).
KV_SCHED = {0: (0, 1), 2: (2, 3, 4, 5), 4: (6, 7, 8, 9), 6: (10, 11, 12, 13),
            1: (14, 15), 3: (), 5: (), 7: ()}

_cache = {}


def _install_ntff_shim():
    # antenv.axon_hooks is absent in this image; register the NTFF profile
    # hook so trace=True can capture HW exec time (harmless if unused).
    if "antenv.axon_hooks" in sys.modules:
        return
    mod = types.ModuleType("antenv.axon_hooks")
    mod._hook = None
    mod.set_axon_ntff_profile_hook = lambda h: setattr(mod, "_hook", h)
    mod.get_axon_ntff_profile_hook = lambda: mod._hook
    sys.modules["antenv.axon_hooks"] = mod
    try:
        import antenv

        antenv.axon_hooks = mod
        from trn_agent_boot.trn_boot import _ntff_profile_via_ctypes

        mod._hook = _ntff_profile_via_ctypes("/opt/axon/libaxon_pjrt.so")
    except Exception:
        pass


def _build():
    import concourse.bass as bass
    import concourse.mybir as mybir
    import concourse.tile as tile
    from concourse import bacc

    F32 = mybir.dt.float32
    F32R = mybir.dt.float32r
    BF16 = mybir.dt.bfloat16
    FP8 = mybir.dt.float8e4
    I32 = mybir.dt.int32
    ADD = mybir.AluOpType.add
    MUL = mybir.AluOpType.mult
    SUB = mybir.AluOpType.subtract
    SHR = mybir.AluOpType.logical_shift_right
    AF = mybir.ActivationFunctionType

    nc = bacc.Bacc("TRN2", target_bir_lowering=False, debug=False, num_devices=NC)

    def din(name, shape, dt=BF16):
        return nc.dram_tensor(name, shape, dt, kind="ExternalInput").ap()

    # host pre-shuffled layouts: partition-major so every DMA is contiguous
    # 4KB+ per partition (8x fewer descriptors than (c p)->p c rearranges).
    xtq = din("xtq", [128, 8, 8, 256])   # [p, qtile, cblk, col]
    xtk = din("xtk", [128, 8, 8, 256])
    xtv = din("xtv", [128, 8, 8, 256])
    wqT = din("wqT", [128, 8, DVC])
    wkT = din("wkT", [128, 8, DVC])
    wvT = din("wvT", [128, 8, DVC])
    woT = din("woT", [128, 8, D])
    smallc = din("smallc", [128, 288], F32)   # bq2|bk2|eps|pad|bv4x64(@16)
    gam_bc = din("gam_bc", [128, D], F32)
    bet_bc = din("bet_bc", [128, D], F32)
    resid = din("resid", [512, D], F32)       # x_q rows + bo (host pre-added)
    mo8_in = din("mo8", [128, 4 * QT], mybir.dt.float8e4)  # diag mask x2 heads
    ones_r = din("ones_r", [1, 64], F32R)
    out_d = nc.dram_tensor("out", [512, D], F32, kind="ExternalOutput").ap()

    groups = [list(range(NC))]

    with nc.allow_low_precision(reason="f32r/bf16 matmul operand chain"), tile.TileContext(
        nc
    ) as tc:
        with (
            tc.tile_pool(name="const", bufs=1) as cpool,
            tc.tile_pool(name="res", bufs=1) as rpool,
            tc.tile_pool(name="xt", bufs=8) as xtpool,
            tc.tile_pool(name="xt5", bufs=2) as xtpool5,
            tc.tile_pool(name="pt", bufs=4) as ptpool,
            tc.tile_pool(name="dn", bufs=3) as dnpool,
            tc.tile_pool(name="gath", bufs=1) as gathpool,
            tc.tile_pool(name="ln", bufs=2) as lnpool,
            tc.tile_pool(name="ps_s", bufs=2, space="PSUM") as pss,
            tc.tile_pool(name="ps_ctx", bufs=2, space="PSUM") as psc,
            tc.tile_pool(name="ps_m", bufs=2, space="PSUM") as psm,
            tc.tile_pool(name="dram", bufs=1, space="DRAM") as dram,
        ):
            # ---- small constants + projection weights (needed first) ----
            smallc_sb = cpool.tile([128, 288], F32)
            mo8_sb = cpool.tile([128, 4 * QT], FP8)
            wq_sb = cpool.tile([128, 8, DVC], BF16)
            wk_sb = cpool.tile([128, 8, DVC], BF16)
            wv_sb = cpool.tile([128, 8, DVC], BF16)
            # spread startup loads across engine queues so issue overlaps
            nc.sync.dma_start(wk_sb[:], wkT)
            nc.scalar.dma_start(wq_sb[:], wqT)
            nc.gpsimd.dma_start(wv_sb[:], wvT)
            nc.sync.dma_start(smallc_sb[:], smallc)
            ones_sb = cpool.tile([1, 64], F32R)
            nc.sync.dma_start(ones_sb[:], ones_r)
            nc.sync.dma_start(mo8_sb[:], mo8_in)
            bq_sb = smallc_sb[:, 0:2]
            bk_sb = smallc_sb[:, 2:4]
            eps_sb = smallc_sb[:, 12:13]
            nbias_sb = smallc_sb[:, 13:14]  # -2.0 exp bias
            bv_sb = smallc_sb[:, 16:16 + DVC]   # bv broadcast (no ones col)

            # ---- resident activation tensors ----
            QT_sb = rpool.tile([128, 2, S], BF16)   # q^T: [dd(2x128), q]
            KT_sb = rpool.tile([128, 2, S], BF16)   # k^T: [dd(2x128), kpos]
            V_sb = rpool.tile([128, S // 128, HPC * (ND + 4)], FP8)
            ctx_sb = rpool.tile([128, 2, S], BF16)  # ctx^T: [dv(2x128), q]
            # ones columns of the V slots (denominator trick), set once;
            # slots are 68 wide (16B-aligned strides for dual-fp8 ldweights):
            # 64 data cols, a ones col, 3 zero pad cols. Data cols are fully
            # written by proj_v before use -- only pad+ones need memset.
            v4 = V_sb[:].rearrange("p c (h x) -> p c h x", x=ND + 4)
            nc.gpsimd.memset(v4[:, :, :, ND:ND + 4], 0.0)
            nc.gpsimd.memset(v4[:, :, :, ND:ND + 1], 1.0)

            # ---- heavyweight phase-3 constants: loaded later (see below) --
            wo_sb = cpool.tile([128, 8, D], BF16)
            gam_sb = cpool.tile([128, D], F32)
            bet_sb = cpool.tile([128, D], F32)
            resp_sb = cpool.tile([128, 4, D], F32)  # all residual chunks

            # ---- A2A buffers ----
            # Row ownership is interleaved at 128-row granularity across BOTH
            # batches: core j owns rows [512*(j//2)+128*(j%2), +128) of each
            # batch (even-tile set, parity 0) plus the same +256 (odd set).
            # Every A2A slot then carries real data -- no batch-dup zeros, no
            # receive-side select -- at half the previous payload.
            a2aA_in = dram.tile([NC, DVC, 128], BF16, name="a2aA_in")
            a2aA_out = dram.tile([NC, DVC, 128], BF16, name="a2aA_out")
            a2aB_in = dram.tile([NC, DVC, 64], BF16, name="a2aB_in")
            a2aB_out = dram.tile([NC, DVC, 64], BF16, name="a2aB_out")
            a2aC1_in = dram.tile([NC, DVC, 32], BF16, name="a2aC1_in")
            a2aC1_out = dram.tile([NC, DVC, 32], BF16, name="a2aC1_out")
            a2aC2_in = dram.tile([NC, DVC, 32], BF16, name="a2aC2_in")
            a2aC2_out = dram.tile([NC, DVC, 32], BF16, name="a2aC2_out")

            def proj_kq(w_sb, xt_d, b_sb, o_sb, c0):
                # project 256 source columns [c0, c0+256) into o_sb (K^T/Q^T)
                xts = xtpool.tile([128, 8, 256], BF16, tag="xt")
                nc.sync.dma_start(xts[:], xt_d[:, c0 // 256, :, :])
                for m in range(2):
                    ps = psm.tile([128, 512], F32, tag="m")
                    for cc in range(8):
                        nc.tensor.matmul(
                            ps[:, 0:256],
                            lhsT=w_sb[:, cc, 128 * m:128 * m + 128],
                            rhs=xts[:, cc, :],
                            start=(cc == 0),
                            stop=(cc == 7),
                        )
                    nc.vector.tensor_scalar(
                        out=o_sb[:, m, c0:c0 + 256],
                        in0=ps[:, 0:256],
                        scalar1=b_sb[:, m:m + 1],
                        scalar2=None,
                        op0=ADD,
                    )

            def proj_kq512(w_sb, xt_d, b_sb, o_sb, c0):
                # project 512 source columns [c0, c0+512) in N=512 matmuls:
                # half the instruction count of two 256-col chunks, better
                # LDWEIGHTS amortization.
                xts = xtpool5.tile([128, 2, 8, 256], BF16, tag="xt5")
                nc.sync.dma_start(xts[:], xt_d[:, c0 // 256:c0 // 256 + 2, :, :])
                for m in range(2):
                    ps = psm.tile([128, 512], F32, tag="m")
                    for cc in range(8):
                        nc.tensor.matmul(
                            ps[:],
                            lhsT=w_sb[:, cc, 128 * m:128 * m + 128],
                            rhs=xts[:, :, cc, :],
                            start=(cc == 0),
                            stop=(cc == 7),
                        )
                    nc.vector.tensor_scalar(
                        out=o_sb[:, m, c0:c0 + 512],
                        in0=ps[:],
                        scalar1=b_sb[:, m:m + 1],
                        scalar2=None,
                        op0=ADD,
                    )

            def proj_v(c0):
                # project V for k rows [c0, c0+256) (two 128-blocks)
                xvs = xtpool.tile([128, 8, 256], BF16, tag="xt")
                nc.sync.dma_start(xvs[:], xtv[:, c0 // 256, :, :])
                for r in range(2):
                    rc = c0 // 128 + r
                    ps = psm.tile([128, 512], F32, tag="m")
                    for cc in range(8):
                        nc.tensor.matmul(
                            ps[:, 0:DVC],
                            lhsT=xvs[:, cc, 128 * r:128 * r + 128],
                            rhs=wv_sb[:, cc, :],
                            start=(cc == 0),
                            stop=(cc == 7),
                        )
                    v_slot = V_sb[:, rc, :].rearrange("p (h x) -> p h x", x=ND + 4)[
                        :, :, 0:ND
                    ]
                    nc.vector.tensor_tensor(
                        out=v_slot,
                        in0=ps[:, 0:DVC].rearrange("p (h x) -> p h x", x=ND),
                        in1=bv_sb.rearrange("p (h x) -> p h x", x=ND),
                        op=ADD,
                    )

            # pt layout per (hc, jp): [128 kpos, hp(2) x u(2) x q(256)].
            # PV for head parity hp: rhs = pt[:, hp, :, :] (contraction over
            # the jp's two 128-blocks via DoubleRow fp8).
            # PSUM banking: start=True clears has_written for the WHOLE bank,
            # so two accumulation chains must never interleave inside one
            # bank. Bank = ctxps_pair[hp]; within it the hc=0 chain fully
            # precedes the hc=1 chain (cols 256*hc) -- sequential per bank.
            def emit_pv(unit):
                hc, jp, pt, ctxps_pair, t = unit
                ptv = pt.rearrange("p (f u q) -> p f u q", u=2, q=256)
                for hp in range(2):
                    h = 2 * hc + hp
                    nc.tensor.matmul(
                        ctxps_pair[hp][0:ND + 4, 256 * hc:256 * hc + 256],
                        lhsT=V_sb[:, 2 * jp:2 * jp + 2, (ND + 4) * h:(ND + 4) * (h + 1)],
                        rhs=ptv[:, hp, :, :],
                        start=(jp == 0),
                        stop=(jp == t),
                        perf_mode=mybir.MatmulPerfMode.DoubleRow,
                        skip_group_check=True,
                    )

            # Deferred per-tile epilogue, run during iteration t+1 so the PE
            # never waits on the reciprocal chain: denominator reciprocal on
            # DVE straight out of PSUM, partition-broadcast on GPSIMD, then
            # the normalize-divides and the ship DMAs of tile t.
            def finish_tile(pend):
                t, ctxps_pair = pend
                dcp = dnpool.tile([1, 1024], F32, tag="dcp")
                dn0 = dnpool.tile([1, 1024], F32, tag="dn0")
                rcp = dnpool.tile([64, 1024], F32, tag="rcp")
                for pi in range(2):
                    # ACT copies PSUM->SBUF (custom-DVE ops cannot read PSUM)
                    nc.scalar.activation(
                        dcp[0:1, 512 * pi:512 * pi + 512],
                        ctxps_pair[pi][64:65, 0:512],
                        AF.Copy,
                    )
                    nc.vector.reciprocal_approx_fast(
                        out=dn0[0:1, 512 * pi:512 * pi + 512],
                        in_=dcp[0:1, 512 * pi:512 * pi + 512],
                    )
                nc.gpsimd.partition_broadcast(rcp[:], dn0[0:1, :], channels=64)
                for h in range(HPC):
                    hp = h % 2
                    hc = h // 2
                    po = 64 * hp
                    co = 256 * hc
                    nc.vector.tensor_tensor(
                        out=ctx_sb[po:po + 64, hc, QT * t:QT * t + QT],
                        in0=ctxps_pair[hp][0:64, co:co + 256],
                        in1=rcp[:, 512 * hp + co:512 * hp + co + 256],
                        op=MUL,
                    )
                if t % 2 == 0:
                    for hh in range(2):
                        nc.sync.dma_start(
                            a2aA_in[t + hh].rearrange("(m p) q -> p m q", p=128),
                            ctx_sb[:, :, QT * t + 128 * hh:QT * t + 128 * hh + 128],
                        )
                elif t in (1, 3):
                    for s2 in range(4):
                        nc.sync.dma_start(
                            a2aB_in[4 * (t == 3) + s2].rearrange(
                                "(m p) q -> p m q", p=128
                            ),
                            ctx_sb[:, :, QT * t + 64 * s2:QT * t + 64 * s2 + 64],
                        )
                else:
                    cx = a2aC1_in if t == 5 else a2aC2_in
                    for s2 in range(8):
                        nc.sync.dma_start(
                            cx[s2].rearrange("(m p) q -> p m q", p=128),
                            ctx_sb[:, :, QT * t + 32 * s2:QT * t + 32 * s2 + 32],
                        )
                trig = {6: (a2aA_in, a2aA_out), 3: (a2aB_in, a2aB_out),
                        5: (a2aC1_in, a2aC1_out), 7: (a2aC2_in, a2aC2_out)}.get(t)
                if trig is not None:
                    nc.gpsimd.collective_compute(
                        "AllToAll",
                        mybir.AluOpType.bypass,
                        replica_groups=groups,
                        ins=[trig[0].opt()],
                        outs=[trig[1].opt()],
                    )

            # ================= main loop =================
            # Attention is emitted as a stream of (hc, jp) units; each unit's
            # PV matmuls are issued one unit later (pending_pv) so the PE's
            # strict-FIFO queue is never parked behind an un-finished Exp.
            pending = None
            pending_pv = None

            def flush_pv():
                nonlocal pending_pv
                if pending_pv is not None:
                    emit_pv(pending_pv)
                    pending_pv = None

            for i, t in enumerate(ITERS):
                # ---- streamed projections for this iteration ----
                blocks = KV_SCHED[t]
                if len(blocks) == 4:
                    proj_kq512(wk_sb, xtk, bk_sb, KT_sb, blocks[0] * 128)
                else:
                    for p0 in range(0, len(blocks), 2):
                        proj_kq(wk_sb, xtk, bk_sb, KT_sb, blocks[p0] * 128)
                proj_kq(wq_sb, xtq, bq_sb, QT_sb, QT * t)
                flush_pv()
                for p0 in range(0, len(blocks), 2):
                    proj_v(blocks[p0] * 128)
                if pending is not None:
                    finish_tile(pending)
                    pending = None
                if i == 4:
                    # phase-3 constants: load mid-flight, off the hot window
                    nc.sync.dma_start(wo_sb[:], woT)
                    nc.sync.dma_start(gam_sb[:], gam_bc)
                    nc.sync.dma_start(bet_sb[:], bet_bc)
                    # prefetch the residual rows now; two of these otherwise
                    # load on the post-collective tail
                    nc.sync.dma_start(
                        resp_sb[:], resid.rearrange("(r p) n -> p r n", p=128)
                    )

                # ---- attention for q-tile t ----
                ctxps_pair = []
                for _pi in range(2):
                    cpt = psc.tile([128, 512], F32, tag="c")
                    ctxps_pair.append(cpt)
                for hc in range(2):
                    for jp in range(t + 1):
                        sps = pss.tile([128, 1024], F32, tag="s")
                        # two heads' scores issued adjacently: different PE
                        # row groups -> they run concurrently.
                        for u in range(2):
                            for hp in range(2):
                                po = 64 * hp
                                nc.tensor.matmul(
                                    sps[:, 512 * hp + 256 * u:512 * hp + 256 * u + 256],
                                    lhsT=KT_sb[
                                        po:po + 64,
                                        hc,
                                        128 * (2 * jp + u):128 * (2 * jp + u) + 128,
                                    ],
                                    rhs=QT_sb[po:po + 64, hc, QT * t:QT * t + QT],
                                    start=True,
                                    stop=True,
                                )
                        pt = ptpool.tile([128, 1024], FP8, tag="pt")
                        # bias -2 keeps exp() under fp8e4 max; it cancels
                        # in softmax (the ones-column denominator sums the
                        # same fp8 values).
                        nc.scalar.activation(
                            pt[:], sps[:], AF.Exp, scale=SCALE, bias=nbias_sb
                        )
                        if jp == t:
                            nc.vector.tensor_tensor(
                                out=pt[:], in0=pt[:], in1=mo8_sb, op=MUL
                            )
                        flush_pv()
                        pending_pv = (hc, jp, pt, ctxps_pair, t)
                pending = (t, ctxps_pair)

            # tile 7: last PV + epilogue (dispatches its AllToAll inside).
            flush_pv()
            finish_tile(pending)

            # ---- phase 3: gather + output projection + residual + LN ----
            # Four AllToAlls land progressively; the Tile scheduler hoists
            # the A/B/C1 passes into the exp-bound late-attention gaps, so
            # only C2's tiny (64KB) flight plus one 64-row pass sit on the
            # critical tail.
            half = 1.5

            def ln_store(y_sb, p0, pn, orow):
                sl = slice(p0, p0 + pn)
                st = lnpool.tile([128, 16], F32, tag="st")
                sti = lnpool.tile([128, 2], I32, tag="sti")
                nc.vector.bn_stats(st[sl, 0:6], y_sb[sl, 0:512])
                nc.vector.bn_stats(st[sl, 6:12], y_sb[sl, 512:1024])
                nc.vector.bn_aggr(st[sl, 12:14], st[sl, 0:12])
                mu = st[sl, 12:13]
                # v = var + eps; y0 = bitcast(0x5f3759df - (v_int >> 1))
                nc.vector.tensor_tensor(
                    out=st[sl, 14:15], in0=st[sl, 13:14], in1=eps_sb[sl], op=ADD
                )
                v = st[sl, 14:15]
                nc.vector.tensor_scalar(
                    out=sti[sl, 0:1], in0=v.bitcast(I32), scalar1=1,
                    scalar2=None, op0=SHR,
                )
                nc.vector.tensor_scalar(
                    out=sti[sl, 1:2], in0=sti[sl, 0:1], scalar1=-1,
                    scalar2=0x5F3759DF, op0=MUL, op1=ADD,
                )
                y0 = sti[sl, 1:2].bitcast(F32)
                # h2 = -0.5 v ; two Newton steps: y <- y*(1.5 + h2*y*y)
                nc.vector.tensor_scalar(
                    out=st[sl, 15:16], in0=v, scalar1=-0.5, scalar2=None, op0=MUL
                )
                h2 = st[sl, 15:16]
                nc.vector.tensor_tensor(out=st[sl, 0:1], in0=y0, in1=y0, op=MUL)
                nc.vector.tensor_scalar(
                    out=st[sl, 1:2], in0=st[sl, 0:1], scalar1=h2, scalar2=half,
                    op0=MUL, op1=ADD,
                )
                nc.vector.tensor_tensor(out=st[sl, 2:3], in0=y0, in1=st[sl, 1:2], op=MUL)
                nc.vector.tensor_tensor(
                    out=st[sl, 3:4], in0=st[sl, 2:3], in1=st[sl, 2:3], op=MUL
                )
                nc.vector.tensor_scalar(
                    out=st[sl, 4:5], in0=st[sl, 3:4], scalar1=h2, scalar2=half,
                    op0=MUL, op1=ADD,
                )
                nc.vector.tensor_tensor(
                    out=st[sl, 5:6], in0=st[sl, 2:3], in1=st[sl, 4:5], op=MUL
                )
                rstd = st[sl, 5:6]
                # yc = (y - mu) * rstd ; out = yc*gamma + beta
                yc = lnpool.tile([128, D], F32, tag="yc")
                nc.vector.scalar_tensor_tensor(
                    out=yc[sl, :], in0=y_sb[sl, :], scalar=mu, in1=gam_sb[sl, :],
                    op0=SUB, op1=MUL,
                )
                nc.vector.scalar_tensor_tensor(
                    out=yc[sl, :], in0=yc[sl, :], scalar=rstd, in1=bet_sb[sl, :],
                    op0=MUL, op1=ADD,
                )
                nc.sync.dma_start(out_d[orow:orow + pn, :], yc[sl, :])

            gathA = gathpool.tile([128, 2, 8, 128], BF16, tag="gathA")
            gathB = gathpool.tile([128, 2, 8, 64], BF16, tag="gathB")
            gathC1 = gathpool.tile([128, 2, 8, 32], BF16, tag="gathC1")
            gathC2 = gathpool.tile([128, 2, 8, 32], BF16, tag="gathC2")
            for gath, grp_out in (
                (gathA, a2aA_out), (gathB, a2aB_out),
                (gathC1, a2aC1_out), (gathC2, a2aC2_out),
            ):
                for bb in range(2):
                    nc.scalar.dma_start(
                        gath[:, bb, :, :].rearrange("p (s m) q -> p s m q", m=2),
                        grp_out[4 * bb:4 * bb + 4].rearrange(
                            "s (m p) q -> p s m q", p=128
                        ),
                    )

            # pass A: two 128-row chunks (one per batch)
            for rc in range(2):
                y_sb = lnpool.tile([128, D], F32, tag="y")
                for n in range(2):
                    ps = psm.tile([128, 512], F32, tag="m")
                    for d2 in range(8):
                        nc.tensor.matmul(
                            ps[:],
                            lhsT=gathA[:, rc, d2, :],
                            rhs=wo_sb[:, d2, 512 * n:512 * n + 512],
                            start=(d2 == 0),
                            stop=(d2 == 7),
                        )
                    nc.vector.tensor_tensor(
                        out=y_sb[:, 512 * n:512 * n + 512],
                        in0=ps[:],
                        in1=resp_sb[:, rc, 512 * n:512 * n + 512],
                        op=ADD,
                    )
                ln_store(y_sb, 0, 128, 128 * rc)

            # pass B: 64-row slots, both batches col-tiled (separate PSUM
            # banks -- bank-wide has_written -- and separate column groups)
            yB = lnpool.tile([128, D], F32, tag="y")
            for n in range(2):
                ps0 = psm.tile([128, 512], F32, tag="m")
                ps1 = psm.tile([128, 512], F32, tag="m")
                for d2 in range(8):
                    nc.tensor.matmul(
                        ps0[0:64, :],
                        lhsT=gathB[:, 0, d2, :],
                        rhs=wo_sb[:, d2, 512 * n:512 * n + 512],
                        start=(d2 == 0), stop=(d2 == 7),
                        skip_group_check=True,
                    )
                    nc.tensor.matmul(
                        ps1[64:128, :],
                        lhsT=gathB[:, 1, d2, :],
                        rhs=wo_sb[:, d2, 512 * n:512 * n + 512],
                        start=(d2 == 0), stop=(d2 == 7),
                        skip_group_check=True,
                    )
                nc.vector.tensor_tensor(
                    out=yB[0:64, 512 * n:512 * n + 512], in0=ps0[0:64, :],
                    in1=resp_sb[0:64, 2, 512 * n:512 * n + 512], op=ADD,
                )
                nc.vector.tensor_tensor(
                    out=yB[64:128, 512 * n:512 * n + 512], in0=ps1[64:128, :],
                    in1=resp_sb[64:128, 2, 512 * n:512 * n + 512], op=ADD,
                )
            ln_store(yB, 0, 128, 256)

            # passes C1/C2: 32-row slots, both batches col-tiled, each pass
            # in its own partition quadrant (C1 at 0-63, C2 at 64-127)
            for gath, po, orow in ((gathC1, 0, 384), (gathC2, 64, 448)):
                yC = lnpool.tile([128, D], F32, tag="y")
                for n in range(2):
                    ps0 = psm.tile([128, 512], F32, tag="m")
                    ps1 = psm.tile([128, 512], F32, tag="m")
                    for d2 in range(8):
                        nc.tensor.matmul(
                            ps0[po:po + 32, :],
                            lhsT=gath[:, 0, d2, :],
                            rhs=wo_sb[:, d2, 512 * n:512 * n + 512],
                            start=(d2 == 0), stop=(d2 == 7),
                            skip_group_check=True,
                            tile_position=(0, po),
                        )
                        nc.tensor.matmul(
                            ps1[po + 32:po + 64, :],
                            lhsT=gath[:, 1, d2, :],
                            rhs=wo_sb[:, d2, 512 * n:512 * n + 512],
                            start=(d2 == 0), stop=(d2 == 7),
                            skip_group_check=True,
                            tile_position=(0, po + 32),
                        )
                    nc.vector.tensor_tensor(
                        out=yC[po:po + 32, 512 * n:512 * n + 512],
                        in0=ps0[po:po + 32, :],
                        in1=resp_sb[po:po + 32, 3, 512 * n:512 * n + 512],
                        op=ADD,
                    )
                    nc.vector.tensor_tensor(
                        out=yC[po + 32:po + 64, 512 * n:512 * n + 512],
                        in0=ps1[po + 32:po + 64, :],
                        in1=resp_sb[po + 32:po + 64, 3, 512 * n:512 * n + 512],
                        op=ADD,
                    )
                ln_store(yC, po, 64, orow)

    nc.compile()
    return nc


def _prep_inputs(x_q, x_k, x_v, mask, Wq, bq, Wk, bk, Wv, bv, Wo, bo, gamma, beta):
    import ml_dtypes

    f = np.float32
    bf = ml_dtypes.bfloat16
    maskA = np.zeros((KB, QT), f)
    maskB = np.zeros((KB, QT), f)
    for i in range(KB):
        maskA[i, i:] = 1.0
        if i + 128 < QT:
            maskB[i, i + 128:] = 1.0
    mo1 = np.concatenate([maskA, maskB], axis=1)
    mo8 = np.concatenate([mo1, mo1], axis=1).astype(ml_dtypes.float8_e4m3)

    def shuf_w(w):  # [D, n] -> [128, 8, n] with row p,c = w[c*128+p]
        return np.ascontiguousarray(
            w.reshape(8, 128, w.shape[1]).transpose(1, 0, 2).astype(bf)
        )

    def shuf_x(x):  # [D, S] -> [128, 8(qtile), 8(cblk), 256]
        # [p, T, c, j] = x[c*128+p, 256*T+j]
        x4 = x.reshape(8, 128, 8, 256)  # [c, p, T, j]
        return np.ascontiguousarray(x4.transpose(1, 2, 0, 3).astype(bf))

    in_maps = []
    for c in range(NC):
        b, g = c // 4, c % 4
        dv = slice(DVC * g, DVC * (g + 1))
        # row ownership (see A2A comment in _build): evens at 128-row,
        # {1,3} at 64-row, tiles 5 and 7 at 32-row granularity.
        re = 512 * (c // 2) + 128 * (c % 2)
        r0B = 256 * (1 if c < 4 else 3) + 64 * (c % 4)
        r5 = 256 * 5 + 32 * c
        r7 = 256 * 7 + 32 * c
        smallc = np.zeros((128, 288), f)
        smallc[:, 0:2] = bq[dv].astype(f).reshape(2, 128).T
        smallc[:, 2:4] = bk[dv].astype(f).reshape(2, 128).T
        smallc[:, 4] = 1.0 - b
        smallc[:, 5] = float(b)
        smallc[:, 12] = EPS
        smallc[:, 13] = -2.0
        smallc[:, 16:16 + DVC] = np.broadcast_to(bv[dv].astype(f), (128, DVC))
        in_maps.append(
            {
                "xtq": shuf_x(x_q[b].T),
                "xtk": shuf_x(x_k[b].T),
                "xtv": shuf_x(x_v[b].T),
                "wqT": shuf_w(Wq[dv, :].T),
                "wkT": shuf_w(Wk[dv, :].T),
                "wvT": shuf_w(Wv[dv, :].T),
                "woT": shuf_w(Wo.T),
                "smallc": smallc,
                "gam_bc": np.broadcast_to(gamma.astype(f), (128, D)).copy(),
                "bet_bc": np.broadcast_to(beta.astype(f), (128, D)).copy(),
                "resid": np.ascontiguousarray(
                    np.concatenate(
                        [
                            x_q[0, re:re + 128, :],
                            x_q[1, re:re + 128, :],
                            x_q[0, r0B:r0B + 64, :],
                            x_q[1, r0B:r0B + 64, :],
                            x_q[0, r5:r5 + 32, :],
                            x_q[1, r5:r5 + 32, :],
                            x_q[0, r7:r7 + 32, :],
                            x_q[1, r7:r7 + 32, :],
                        ]
                    ).astype(f)
                    + bo.astype(f)
                ),
                "mo8": mo8,
                "ones_r": np.ones((1, 64), f),
            }
        )
    return in_maps


def kernel(x_q, x_k, x_v, mask, Wq, bq, Wk, bk, Wv, bv, Wo, bo, gamma, beta):
    _install_ntff_shim()
    from concourse.bass_utils import run_bass_kernel_spmd

    x_q, x_k, x_v = np.asarray(x_q), np.asarray(x_k), np.asarray(x_v)
    mask = np.asarray(mask)
    # this kernel implements causal attention structurally; verify the mask
    causal = np.tril(np.ones((S, S), mask.dtype))
    assert np.array_equal(mask.reshape(S, S), causal), "kernel specialized for causal mask"

    if "nc" not in _cache:
        _cache["nc"] = _build()
    nc = _cache["nc"]

    in_maps = _prep_inputs(
        x_q, x_k, x_v, mask,
        np.asarray(Wq), np.asarray(bq), np.asarray(Wk), np.asarray(bk),
        np.asarray(Wv), np.asarray(bv), np.asarray(Wo), np.asarray(bo),
        np.asarray(gamma), np.asarray(beta),
    )
    res = run_bass_kernel_spmd(nc, in_maps, list(range(NC)))
    _cache["last_results"] = res

    out = np.empty((B, S, D), np.float32)
    for c in range(NC):
        re = 512 * (c // 2) + 128 * (c % 2)
        r0B = 256 * (1 if c < 4 else 3) + 64 * (c % 4)
        r5 = 256 * 5 + 32 * c
        r7 = 256 * 7 + 32 * c
        r = res.results[c]["out"]
        out[0, re:re + 128, :] = r[0:128]
        out[1, re:re + 128, :] = r[128:256]
        out[0, r0B:r0B + 64, :] = r[256:320]
        out[1, r0B:r0B + 64, :] = r[320:384]
        out[0, r5:r5 + 32, :] = r[384:416]
        out[1, r5:r5 + 32, :] = r[416:448]
        out[0, r7:r7 + 32, :] = r[448:480]
        out[1, r7:r7 + 32, :] = r[480:512]
    return out
